# revision 1
# baseline (speedup 1.0000x reference)
"""Causal multi-head attention on 8 Trainium2 NeuronCores.

Sharding: core c -> (batch b = c//2, head-group g = c%2 of 6 heads).
Each core computes its 6 heads' attention output contracted through its
slice of W_O; the two half-head partial outputs per batch are summed on
the host (no device collectives), and b_O is added on the host.

Device algorithm per core (all matmuls fp32r, full PE rate at N>=256):
  - host supplies xT = x[b].T  [768, 2048] so no on-device transpose
  - QT/KT [384, 2048] = (wq|wk).T @ xT     (heads stacked on partitions)
  - V [2048, 6*65] natural layout, with a ones-column per head so the
    softmax denominator falls out of the PV matmul as row 64
  - S^T tiles [128k, 512q] = KT_head-slice.T @ QT_head (k on partitions;
    two heads packed in the PE array via row groups 0-63 / 64-127)
  - causal mask = additive -1e6 triangle on the PSUM S tile (DVE),
    before exp, so exp output feeds the PV matmul directly
  - e = exp(S^T / 8) on ACT (batched [128,1024] over kt pairs)
  - z^T[65, 512] += V_aug.T @ e  accumulated over k tiles in PSUM
  - zhat = z * (1/denom), denom broadcast via a K=1 ones matmul
  - out[128q, 768] += zhat_pair.T @ wo_pair  accumulated over 3 pairs
Phase 1 runs chunk-outer (q-chunk granularity) with split DMAs across
two DGE queues so attention starts while projections are still running.
"""

import numpy as np

B = 4
S = 2048
D = 768
NH = 12
DH = 64
G = 2            # head groups (tensor-parallel)
HPG = NH // G    # heads per group = 6
NP = HPG // 2    # head pairs per group = 3
KT = D // 128    # 6 k-tiles of the d_model contraction
ST = S // 128    # 16 s-tiles
QC = S // 512    # 4 q-chunks
N_CORES = 8


def _split_drain_waits(nc, mybir, max_waits=1):
    """This container's walrus only accepts one sync wait per instruction;
    hoist extra waits onto preceding single-wait NoOps on the same engine
    (engines execute in program order, so the waits still gate the inst)."""
    for f in nc.m.functions:
        for bb in f.blocks:
            newlist = []
            for ins in bb.instructions:
                si = ins.sync_info
                if si is not None and si.on_wait and len(si.on_wait) > max_waits:
                    waits = list(si.on_wait)
                    for i, w in enumerate(waits[:-max_waits]):
                        d = mybir.InstNoOp(name=f"{ins.name}-sw{i}", ins=[], outs=[])
                        d.engine = ins.engine
                        d.sync_info = mybir.SyncInfo(on_wait=[w], on_update=[])
                        newlist.append(d)
                    ins.sync_info = mybir.SyncInfo(
                        on_wait=list(waits[-max_waits:]), on_update=list(si.on_update)
                    )
                newlist.append(ins)
            try:
                bb.instructions = newlist
            except Exception:
                bb.instructions.clear()
                bb.instructions.extend(newlist)


def build_program(reps=1):
    import concourse.bass as bass
    import concourse.mybir as mybir
    import concourse.tile as tile

    f32 = mybir.dt.float32
    f32r = mybir.dt.float32r
    EXP = mybir.ActivationFunctionType.Exp
    IDENT = mybir.ActivationFunctionType.Identity

    nc = bass.Bass("TRN2")
    xT = nc.dram_tensor("xT", [D, S], f32, kind="ExternalInput")
    wq = nc.dram_tensor("wq", [D, HPG * DH], f32, kind="ExternalInput")
    wk = nc.dram_tensor("wk", [D, HPG * DH], f32, kind="ExternalInput")
    wv = nc.dram_tensor("wv", [D, HPG * DH], f32, kind="ExternalInput")
    wo = nc.dram_tensor("wo", [HPG * DH, D], f32, kind="ExternalInput")
    bq = nc.dram_tensor("bq", [HPG * DH], f32, kind="ExternalInput")
    bk = nc.dram_tensor("bk", [HPG * DH], f32, kind="ExternalInput")
    bv = nc.dram_tensor("bv", [1, HPG * DH], f32, kind="ExternalInput")
    m01 = nc.dram_tensor("m01", [128, 512], f32, kind="ExternalInput")
    ones_d = nc.dram_tensor("ones_d", [1, 128], f32, kind="ExternalInput")
    ones2_d = nc.dram_tensor("ones2_d", [128, 128], f32, kind="ExternalInput")
    out = nc.dram_tensor("out", [S, D], f32, kind="ExternalOutput")

    HD = HPG * DH  # 384

    from contextlib import ExitStack

    with tile.TileContext(nc) as tc:
        with ExitStack() as _ctx:
            _e = _ctx.enter_context
            _e(nc.allow_low_precision(reason="fp32r matmul pipeline"))
            wpool = _e(tc.tile_pool(name="weights", bufs=1))
            xtpool = _e(tc.tile_pool(name="xt", bufs=KT))
            qtpool = _e(tc.tile_pool(name="qt", bufs=NP * QC))
            ktpool = _e(tc.tile_pool(name="kt", bufs=NP * QC))
            vpool = _e(tc.tile_pool(name="v", bufs=ST))
            epool = _e(tc.tile_pool(name="e", bufs=3))
            smallpool = _e(tc.tile_pool(name="small", bufs=2))
            zhpool = _e(tc.tile_pool(name="zh", bufs=1))
            zupool = _e(tc.tile_pool(name="zu", bufs=1))
            dnpool = _e(tc.tile_pool(name="dn", bufs=1))
            opool = _e(tc.tile_pool(name="osb", bufs=1))
            pspool = _e(tc.tile_pool(name="ps", bufs=2, space="PSUM"))
            pzpool = _e(tc.tile_pool(name="pz", bufs=2, space="PSUM"))
            popool = _e(tc.tile_pool(name="po", bufs=1, space="PSUM"))

            # ---- small constants first (block nothing) ----
            ones_sb = wpool.tile([1, 128], f32r, tag="ones")
            nc.sync.dma_start(ones_sb[:], ones_d[:].bitcast(f32r))
            ones2_sb = wpool.tile([128, 128], f32r, tag="ones2")
            nc.sync.dma_start(ones2_sb[:], ones2_d[:].bitcast(f32r))
            bq_sb = wpool.tile([128, NP], f32, tag="bq")
            nc.sync.dma_start(bq_sb[:], bq[:].rearrange("(j p) -> p j", p=128))
            bk_sb = wpool.tile([128, NP], f32, tag="bk")
            nc.sync.dma_start(bk_sb[:], bk[:].rearrange("(j p) -> p j", p=128))
            bv_sb = wpool.tile([1, HD], f32r, tag="bv")
            nc.gpsimd.dma_start(bv_sb[:], bv[:].bitcast(f32r))
            m01_sb = wpool.tile([128, 512], f32r, tag="m01")
            nc.gpsimd.dma_start(m01_sb[:], m01[:].bitcast(f32r))

            # ---- weights and activations, split per k-tile block over two
            # DGE queues so the first projection matmuls start early ----
            wq_sb = wpool.tile([128, KT * HD], f32r, tag="wq")
            wk_sb = wpool.tile([128, KT * HD], f32r, tag="wk")
            wv_sb = wpool.tile([128, KT * HD], f32r, tag="wv")
            xt_sb = [xtpool.tile([128, S], f32r, tag="xt", name=f"xt{i}") for i in range(KT)]
            for a in range(KT):
                asl = slice(a * 128, (a + 1) * 128)
                nc.sync.dma_start(
                    wq_sb[:, a * HD : (a + 1) * HD], wq[asl, :].bitcast(f32r)
                )
                nc.gpsimd.dma_start(
                    wk_sb[:, a * HD : (a + 1) * HD], wk[asl, :].bitcast(f32r)
                )
            for c in range(QC):
                csl = slice(c * 512, (c + 1) * 512)
                for a in range(KT):
                    eng = nc.sync if a % 2 == 0 else nc.gpsimd
                    eng.dma_start(
                        xt_sb[a][:, csl],
                        xT[a * 128 : (a + 1) * 128, csl].bitcast(f32r),
                    )
                if c == 0:
                    for a in range(KT):
                        eng = nc.sync if a % 2 == 1 else nc.gpsimd
                        eng.dma_start(
                            wv_sb[:, a * HD : (a + 1) * HD],
                            wv[a * 128 : (a + 1) * 128, :].bitcast(f32r),
                        )
            wo_sb = wpool.tile([128, NP * D], f32r, tag="wo")
            nc.sync.dma_start(
                wo_sb[:].rearrange("p (j d) -> p j d", j=NP),
                wo[:].bitcast(f32r).rearrange("(j p) d -> p j d", p=128),
            )

            for _rep in range(reps):
                # ---- phase 1: projections, q-chunk outer ----
                qt_sb = [
                    [qtpool.tile([128, 512], f32r, tag="qt", name=f"qt{i}_{cc}")
                     for cc in range(QC)]
                    for i in range(NP)
                ]
                kt_sb = [
                    [ktpool.tile([128, 512], f32r, tag="kt", name=f"kt{i}_{cc}")
                     for cc in range(QC)]
                    for i in range(NP)
                ]
                v_sb = [vpool.tile([128, HPG * 65], f32r, tag="v", name=f"v{i}") for i in range(ST)]
                for c in range(QC):
                    csl = slice(c * 512, (c + 1) * 512)
                    for j in range(NP):
                        for dst, w_sb, b_sb in (
                            (qt_sb[j][c], wq_sb, bq_sb),
                            (kt_sb[j][c], wk_sb, bk_sb),
                        ):
                            ps = pspool.tile([128, 512], f32, tag="ps")
                            for a in range(KT):
                                nc.tensor.matmul(
                                    ps[:],
                                    w_sb[:, a * HD + j * 128 : a * HD + (j + 1) * 128],
                                    xt_sb[a][:, csl],
                                    start=(a == 0),
                                    stop=(a == KT - 1),
                                )
                            # PSUM->SBUF copy + per-partition bias on ACT
                            nc.scalar.activation(
                                dst[:], ps[:], IDENT, bias=b_sb[:, j : j + 1]
                            )
                    # V for the 4 s-tiles of this chunk
                    for st in range(4 * c, 4 * c + 4):
                        vt = v_sb[st]
                        pv = pspool.tile([128, HD + HPG], f32, tag="ps")
                        for a in range(KT):
                            nc.tensor.matmul(
                                pv[:, 0:HD],
                                xt_sb[a][:, st * 128 : (st + 1) * 128],
                                wv_sb[:, a * HD : (a + 1) * HD],
                                start=(a == 0),
                                stop=False,
                            )
                        nc.tensor.matmul(
                            pv[:, 0:HD], ones_sb[:, :], bv_sb[:, :],
                            start=False, stop=True,
                        )
                        # ones columns for the softmax denominator (z row 64)
                        nc.tensor.matmul(
                            pv[:, HD : HD + HPG],
                            ones_sb[:, :], ones_sb[:, 0:HPG],
                            start=True, stop=True,
                        )
                        vtv = vt[:].rearrange("p (n c) -> p n c", n=HPG)
                        nc.scalar.copy(
                            vtv[:, :, 0:DH],
                            pv[:, 0:HD].rearrange("p (n c) -> p n c", n=HPG),
                        )
                        nc.scalar.copy(
                            vtv[:, :, DH : DH + 1],
                            pv[:, HD : HD + HPG].rearrange("p (n c) -> p n c", n=HPG),
                        )

                    # ---- attention for this q-chunk (interleaved) ----
                    klim = 4 * (c + 1)
                    qsl = slice(c * 512, (c + 1) * 512)
                    zh = zhpool.tile([128, NP * 512], f32r, tag="zh")
                    zu = zupool.tile([128, NP * 512], f32, tag="zu")
                    dna = dnpool.tile([128, 512], f32r, tag="dna")
                    dnb = dnpool.tile([128, 512], f32r, tag="dnb")
                    for j in range(NP):
                        pzA = pzpool.tile([65, 512], f32, tag="pz")
                        pzB = pzpool.tile([65, 512], f32, tag="pz")
                        for kp in range(klim // 2):
                            psA = pspool.tile([128, 1024], f32, tag="ps")
                            psB = pspool.tile([128, 1024], f32, tag="ps")
                            for half in range(2):
                                kt_i = 2 * kp + half
                                kc, ko = kt_i // 4, (kt_i % 4) * 128
                                ksl = slice(ko, ko + 128)
                                hsl = slice(half * 512, (half + 1) * 512)
                                nc.tensor.matmul(
                                    psA[:, hsl],
                                    kt_sb[j][kc][0:64, ksl],
                                    qt_sb[j][c][0:64, :],
                                    start=True, stop=True,
                                )
                                nc.tensor.matmul(
                                    psB[:, hsl],
                                    kt_sb[j][kc][64:128, ksl],
                                    qt_sb[j][c][64:128, :],
                                    start=True, stop=True,
                                )
                            eA = epool.tile([128, 1024], f32r, tag="e")
                            eB = epool.tile([128, 1024], f32r, tag="e")
                            nc.scalar.activation(eA[:], psA[:], EXP, scale=0.125)
                            nc.scalar.activation(eB[:], psB[:], EXP, scale=0.125)
                            for half in range(2):
                                kt_i = 2 * kp + half
                                r2 = kt_i - 4 * c
                                if r2 >= 0:
                                    # causal 0/1 mask post-exp (PE absorbs delay)
                                    delta = 128 * r2
                                    wlen = delta + 128
                                    moff = half * 512
                                    for ei, eX in enumerate((eA, eB)):
                                        eng = nc.vector
                                        eng.tensor_mul(
                                            eX[:, moff : moff + wlen],
                                            eX[:, moff : moff + wlen],
                                            m01_sb[:, 384 - delta : 512],
                                        )
                            for half in range(2):
                                kt_i = 2 * kp + half
                                hsl = slice(half * 512, (half + 1) * 512)
                                nc.tensor.matmul(
                                    pzA[:],
                                    v_sb[kt_i][:, (2 * j) * 65 : (2 * j + 1) * 65],
                                    eA[:, hsl],
                                    start=(kt_i == 0),
                                    stop=(kt_i == klim - 1),
                                )
                                nc.tensor.matmul(
                                    pzB[:],
                                    v_sb[kt_i][:, (2 * j + 1) * 65 : (2 * j + 2) * 65],
                                    eB[:, hsl],
                                    start=(kt_i == 0),
                                    stop=(kt_i == klim - 1),
                                )
                        # stage z and denominators in SBUF, release PSUM
                        jsl = slice(j * 512, (j + 1) * 512)
                        nc.vector.tensor_copy(zu[0:64, jsl], pzA[0:64, :])
                        nc.vector.tensor_copy(zu[64:128, jsl], pzB[0:64, :])
                        for which, pz in ((0, pzA), (1, pzB)):
                            i = 2 * j + which
                            dnt = dna if i < 3 else dnb
                            row = 32 * (i % 3)
                            nc.vector.tensor_copy(
                                dnt[row : row + 1, :], pz[64:65, :]
                            )
                    # batched reciprocals in place (rows on 32-aligned partitions)
                    rra, rrb = dna, dnb
                    nc.vector.reciprocal(dna[0:96, :], dna[0:96, :])
                    nc.vector.reciprocal(dnb[0:96, :], dnb[0:96, :])
                    for j in range(NP):
                        jsl = slice(j * 512, (j + 1) * 512)
                        bc_sb = smallpool.tile([128, 512], f32r, tag="bcs")
                        for which in (0, 1):
                            i = 2 * j + which
                            rrt = rra if i < 3 else rrb
                            row = 32 * (i % 3)
                            bc = pzpool.tile([64, 512], f32, tag="pz")
                            nc.tensor.matmul(
                                bc[:], ones2_sb[row : row + 1, 0:64],
                                rrt[row : row + 1, :],
                                start=True, stop=True,
                            )
                            nc.vector.tensor_copy(
                                bc_sb[64 * which : 64 * (which + 1), :], bc[:]
                            )
                        for which in (0, 1):
                            hsl2 = slice(64 * which, 64 * (which + 1))
                            nc.vector.tensor_mul(
                                zh[hsl2, jsl],
                                zu[hsl2, jsl].bitcast(f32r),
                                bc_sb[hsl2, :],
                            )
                    # W_O contraction for this q-chunk
                    for qs in range(4):
                        po = popool.tile([128, D], f32, tag="po")
                        for j in range(NP):
                            lhs = zh[:, j * 512 + qs * 128 : j * 512 + (qs + 1) * 128]
                            nc.tensor.matmul(
                                po[:, 0:512], lhs, wo_sb[:, j * D : j * D + 512],
                                start=(j == 0), stop=(j == NP - 1),
                            )
                            nc.tensor.matmul(
                                po[:, 512:768], lhs,
                                wo_sb[:, j * D + 512 : (j + 1) * D],
                                start=(j == 0), stop=(j == NP - 1),
                            )
                        osb = opool.tile([128, D], f32, tag="osb")
                        nc.vector.tensor_copy(osb[:], po[:])
                        row = c * 512 + qs * 128
                        nc.sync.dma_start(out[row : row + 128, :], osb[:])

    _split_drain_waits(nc, mybir)
    return nc


_nc_cache = None


def kernel(normalized_resid_pre, W_Q, W_K, W_V, W_O, b_Q, b_K, b_V, b_O):
    from concourse.bass_utils import run_bass_kernel_spmd

    global _nc_cache
    if _nc_cache is None:
        _nc_cache = build_program()
    nc = _nc_cache

    x = np.asarray(normalized_resid_pre, np.float32)

    # additive causal mask: m01[p, u] = 0 where p <= u-384 else -1e6.
    # Boundary S^T tile with row offset delta uses slice [384-delta : 512).
    p = np.arange(128)[:, None]
    u = np.arange(512)[None, :]
    m01 = np.where(p <= u - 384, 1.0, 0.0).astype(np.float32)

    in_maps = []
    for c in range(N_CORES):
        b, g = c // G, c % G
        hs = slice(g * HPG, (g + 1) * HPG)
        in_maps.append(
            {
                "xT": np.ascontiguousarray(x[b].T),
                "wq": np.ascontiguousarray(
                    W_Q[hs].transpose(1, 0, 2).reshape(D, HPG * DH)
                ),
                "wk": np.ascontiguousarray(
                    W_K[hs].transpose(1, 0, 2).reshape(D, HPG * DH)
                ),
                "wv": np.ascontiguousarray(
                    W_V[hs].transpose(1, 0, 2).reshape(D, HPG * DH)
                ),
                "wo": np.ascontiguousarray(W_O[hs].reshape(HPG * DH, D)),
                "bq": np.ascontiguousarray(b_Q[hs].reshape(-1)),
                "bk": np.ascontiguousarray(b_K[hs].reshape(-1)),
                "bv": np.ascontiguousarray(b_V[hs].reshape(1, -1)),
                "m01": m01,
                "ones_d": np.ones((1, 128), np.float32),
                "ones2_d": np.ones((128, 128), np.float32),
            }
        )

    res = run_bass_kernel_spmd(nc, in_maps, core_ids=list(range(N_CORES)))
    out = np.zeros((B, S, D), np.float32)
    for c in range(N_CORES):
        out[c // G] += res.results[c]["out"]
    out += np.asarray(b_O, np.float32)
    return out

